# revision 2
# baseline (speedup 1.0000x reference)
"""FP8-quantized dense MLP (scaled matmul) on 8 Trainium2 NeuronCores.

Reference computation:
    x  [8, 2048, 4096] f32, weight [4096, 4096] f32
    sx = 448 / amax(|x|); sw = 448 / amax(|w|)
    out = (q8(x*sx) @ q8(w*sw)) * (1/sx) * (1/sw)     (q8 = OCP e4m3fn RNE)

Sharding: 4 M-shards x 2 N-shards over 8 cores (core c -> rows
[c//2*4096, +4096), cols [c%2*2048, +2048)).  Scales + fp8 quantization run
on host (O(MK+KN) elementwise prep); the O(MKN) matmul runs on device.

TRN2's FP8_EXP4 has max +-240 (OCP e4m3fn has +-448), so OCP-quantized values
256..448 would be NaN/Inf on device.  We therefore quantize to the OCP grid
*halved* (exact in fp8 for all but deep-subnormal values) by scaling with
sx/2 and clipping to +-224, and compensate with a *4 factor folded into the
output scale.  The device matmul (fp8 products, f32 accumulate) is then
bit-equivalent to the reference modulo f32 summation order.

Device kernel per core: out[4096, 2048] = xT.T @ w in fp8 DoubleRow mode
(K-tiles of 256), 216ns/MM warm = the N=512 DoubleRow stream roofline.
Perf structure (from trace analysis of the 467us baseline):
  exec = T_first_MM + 2048*216ns + prologue stalls/cold + tail.
So the optimizations here all attack the edges:
  - DRAM layouts are k2-major (x) / n-major (w) so a small critical prefix
    [w0 n-slice0 128KB, x0 k2-slices 0..XA 128KB] lands first and the first
    real MM issues at ~8.5us instead of 13.1us (input DMA is one ~330GB/s
    queue; program order == arrival order).
  - x1 streams before w1..w15 so during the w-stream phase the PE has
    7 MMs (m0 4 + m1 3) per 512KB w-tile = 1.51us work per 1.51us wire
    cadence: saturated as soon as w1 lands.  (m1's ps3-bank MMs defer
    until m0's ps3 evicts - ps3 is single-buffered - the static scheduler
    handles this, observed in the baseline trace.)
  - Dummy warm-up/filler matmuls (N=256 on a dedicated PSUM bank) are
    emitted between the early MM groups so the PE never idles: idle windows
    re-throttle the HAM clock gate to 1.2GHz (the baseline lost ~5us to a
    mid-prologue re-throttle + cold matmuls).
  - The last m-tile runs n-outer/k2-inner so banks ps0..ps2 finish, evict,
    and DMA out under the remaining MM stream; only ps3's eviction+256KB
    DMA trail the last MM.
"""

import numpy as np
import ml_dtypes

FP8_MAX = 448.0
B, S, K, N = 8, 2048, 4096, 4096
NCORES = 8
MSHARDS = 4
NSHARDS = 2
M_CORE = B * S // MSHARDS   # 4096 rows per core
N_CORE = N // NSHARDS       # 2048 cols per core
P = 128
K2 = K // 256    # 16 DoubleRow k-tiles of 256
MT = M_CORE // P  # 32 m-tiles per core
NFREE = 512      # matmul free dim == one PSUM bank of f32
NT = N_CORE // NFREE  # 4 PSUM banks per m-tile

XA = 4           # k2-slices in the early x0a fragment (XA*32KB)

# Filler counts (N=256 dummy matmuls, ~107ns warm / ~213ns cold each):
# F0 before the stream (absorbs HAM ramp during the first DMAs), F1..F4
# bridge the prologue stalls while w0-rest/x1/x0b/w1 are on the wire.
F0 = 12
F1 = 7
F2 = 9
F3 = 8
F4 = 3

_E4M3 = ml_dtypes.float8_e4m3  # TRN semantics: max +-240

_nc_cache = None


def _build_nc():
    from concourse import bacc, tile, mybir

    nc = bacc.Bacc("TRN2", debug=False)
    xt_d = nc.dram_tensor("xt", [MT, P, K2, 2, P], mybir.dt.float8e4, kind="ExternalInput")
    wt_d = nc.dram_tensor(
        "wt", [K2, P, NT, 2, NFREE], mybir.dt.float8e4, kind="ExternalInput"
    )
    sc_d = nc.dram_tensor("sc", [P, 1], mybir.dt.float32, kind="ExternalInput")
    out_d = nc.dram_tensor("out", [M_CORE, N_CORE], mybir.dt.float32, kind="ExternalOutput")

    with tile.TileContext(nc) as tc:
        with (
            tc.tile_pool(name="wp", bufs=1) as wp,
            tc.tile_pool(name="xp", bufs=4) as xp,
            tc.tile_pool(name="op", bufs=4) as op,
            tc.tile_pool(name="cp", bufs=1) as cp,
            tc.tile_pool(name="pp", bufs=2, space="PSUM") as pp,
        ):
            # Warm-up/filler matmuls get their OWN PSUM bank (tag ps3 drops
            # to one slot below) - sharing a bank with a live accumulator
            # tag crashes the device (PSUM_COLLISION).
            wa = cp.tile([P, 2, P], mybir.dt.float8e4, tag="wa")
            wb = cp.tile([P, 2, 2 * P], mybir.dt.float8e4, tag="wb")
            nc.vector.memset(wa[:], 0)
            nc.vector.memset(wb[:], 0)
            psw = pp.tile([P, 2 * P], mybir.dt.float32, tag="psw", bufs=1, name="psw")

            def filler(count):
                for _ in range(count):
                    nc.tensor.matmul(
                        psw[:],
                        wa[:],
                        wb[:],
                        start=True,
                        stop=True,
                        perf_mode=mybir.MatmulPerfMode.DoubleRow,
                    )

            filler(F0)

            sc_sb = cp.tile([P, 1], mybir.dt.float32, tag="sc")

            # Prologue tiles: w0 split n0 / n123, x0 split k2<XA / rest.
            w0a = wp.tile([P, 2, NFREE], mybir.dt.float8e4, tag="w0a")
            w0r = wp.tile([P, NT - 1, 2, NFREE], mybir.dt.float8e4, tag="w0r")
            x0a = cp.tile([P, XA, 2, P], mybir.dt.float8e4, tag="x0a")
            x0b = cp.tile([P, K2 - XA, 2, P], mybir.dt.float8e4, tag="x0b")
            x_tiles = {1: xp.tile([P, K2, 2, P], mybir.dt.float8e4, tag="x", name="x1")}

            # Input DMAs share one queue: program order == arrival order.
            # Critical prefix first (w0n0+x0a gate the first MM), then the
            # pieces in the order the PE needs them; all DMA triggers stay
            # on nc.sync (gpsimd routing measured a 95us regression in a
            # previous session).
            nc.sync.dma_start(w0a[:], wt_d[0][:, 0])
            nc.sync.dma_start(x0a[:], xt_d[0][:, 0:XA])
            nc.sync.dma_start(w0r[:], wt_d[0][:, 1:NT])
            nc.sync.dma_start(x_tiles[1][:], xt_d[1])
            nc.sync.dma_start(x0b[:], xt_d[0][:, XA:K2])

            w_sb = {}
            for k2 in range(1, K2):
                w_t = wp.tile([P, NT, 2, NFREE], mybir.dt.float8e4, tag=f"w{k2}")
                nc.sync.dma_start(w_t[:], wt_d[k2])
                w_sb[k2] = w_t
                if k2 == 8:
                    nc.sync.dma_start(sc_sb[:], sc_d[:])

            def rhs(k2, n):
                if k2 == 0:
                    return w0a[:] if n == 0 else w0r[:, n - 1]
                return w_sb[k2][:, n]

            def alloc_ps(m):
                # ps3 single-buffered: its double-buffer slot is the
                # warm-up bank (PSUM holds exactly 8 banks; evictions are
                # ~20x faster than an m-tile, so one tag without WAR slack
                # costs little - the scheduler defers the next m-tile's ps3
                # MMs until the eviction, observed working in the trace)
                return [
                    pp.tile(
                        [P, NFREE],
                        mybir.dt.float32,
                        tag=f"ps{n}",
                        name=f"ps{m}_{n}",
                        bufs=1 if n == NT - 1 else 2,
                    )
                    for n in range(NT)
                ]

            def evict_one(m, n, bank):
                o_t = op.tile([P, NFREE], mybir.dt.float32, tag="o", name=f"o{m}_{n}")
                if n % 2 == 0:
                    nc.scalar.activation(
                        o_t[:],
                        bank[:],
                        mybir.ActivationFunctionType.Copy,
                        scale=sc_sb[:],
                    )
                else:
                    nc.vector.tensor_scalar_mul(o_t[:], bank[:], sc_sb[:])
                nc.sync.dma_start(
                    out_d[m * P : (m + 1) * P, n * NFREE : (n + 1) * NFREE],
                    o_t[:],
                )

            for m in range(MT):
                if m == 0:
                    lhs = lambda k2: x0a[:, k2] if k2 < XA else x0b[:, k2 - XA]
                else:
                    if m in x_tiles:
                        x_t = x_tiles.pop(m)
                    else:
                        x_t = xp.tile([P, K2, 2, P], mybir.dt.float8e4, tag="x", name=f"x{m}")
                        nc.sync.dma_start(x_t[:], xt_d[m])
                    lhs = lambda k2, x_t=x_t: x_t[:, k2]
                ps = alloc_ps(m)
                if m < MT - 1:
                    for k2 in range(K2):
                        for n in range(NT):
                            nc.tensor.matmul(
                                ps[n][:],
                                lhs(k2),
                                rhs(k2, n),
                                start=(k2 == 0),
                                stop=(k2 == K2 - 1),
                                perf_mode=mybir.MatmulPerfMode.DoubleRow,
                            )
                            if m == 0 and k2 == 0 and n == 0:
                                filler(F1)
                        if m == 0 and k2 == 0:
                            filler(F2)
                        elif m == 0 and k2 == 1:
                            filler(F3)
                        elif m == 0 and k2 == 2:
                            filler(F4)
                    for n in range(NT):
                        evict_one(m, n, ps[n])
                else:
                    # Last m-tile n-outer: each bank finishes, evicts and
                    # DMAs out under the remaining banks' MM stream.
                    for n in range(NT):
                        for k2 in range(K2):
                            nc.tensor.matmul(
                                ps[n][:],
                                lhs(k2),
                                rhs(k2, n),
                                start=(k2 == 0),
                                stop=(k2 == K2 - 1),
                                perf_mode=mybir.MatmulPerfMode.DoubleRow,
                            )
                        evict_one(m, n, ps[n])

    nc.finalize()
    return nc


def _get_nc():
    global _nc_cache
    if _nc_cache is None:
        _nc_cache = _build_nc()
    return _nc_cache


def _amax(a):
    # max(|a|) without a full |a| temp; exact (max/min are exact in f32)
    return np.float32(max(np.float32(a.max()), -np.float32(a.min())))


def _prep(x, weight):
    """Host prep: scales, halved OCP-grid fp8 quantization, tiled layouts."""
    x = np.asarray(x, dtype=np.float32)
    weight = np.asarray(weight, dtype=np.float32)

    sx = np.float32(FP8_MAX) / np.maximum(_amax(x), np.float32(1e-12))
    sw = np.float32(FP8_MAX) / np.maximum(_amax(weight), np.float32(1e-12))
    clip = np.float32(FP8_MAX / 2.0)  # 224

    # weight: [K, N] -> per N-shard [K2, P, NT, 2, NFREE]:
    #   wt[k2, ki, n, o, f] = wq[k2*256 + o*128 + ki, nh*N_CORE + n*512 + f]
    wbuf = weight * (sw * np.float32(0.5))
    np.clip(wbuf, -clip, clip, out=wbuf)
    wq = wbuf.astype(_E4M3)
    wts = [
        np.ascontiguousarray(
            wq[:, nh * N_CORE : (nh + 1) * N_CORE]
            .reshape(K2, 2, P, NT, NFREE)
            .transpose(0, 2, 3, 1, 4)
        )
        for nh in range(NSHARDS)
    ]

    # x per M-shard ms: rows [ms*4096, +4096) -> [MT, P, K2, 2, P] with
    # xt[m, ki, k2, o, j] = xq[m*128+j, k2*256 + o*128 + ki]
    x2 = x.reshape(B * S, K)
    xts = []
    for ms in range(MSHARDS):
        xbuf = x2[ms * M_CORE : (ms + 1) * M_CORE] * (sx * np.float32(0.5))
        np.clip(xbuf, -clip, clip, out=xbuf)
        xq = xbuf.astype(_E4M3)
        xts.append(
            np.ascontiguousarray(xq.reshape(MT, P, K2, 2, P).transpose(0, 4, 2, 3, 1))
        )

    # output scale: psum = ref_matmul / 4  ->  multiply by 4 * (1/sx) * (1/sw)
    c = np.float32(4.0) * (np.float32(1.0) / sx) * (np.float32(1.0) / sw)
    sc = np.full((P, 1), c, dtype=np.float32)
    return xts, wts, sc


def _run(x, weight, trace=False, tmpdir=None):
    from concourse.bass_utils import run_bass_kernel_spmd

    nc = _get_nc()
    xts, wts, sc = _prep(x, weight)
    in_maps = [
        {"xt": xts[c // NSHARDS], "wt": wts[c % NSHARDS], "sc": sc}
        for c in range(NCORES)
    ]
    res = run_bass_kernel_spmd(
        nc, in_maps, list(range(NCORES)), trace=trace, tmpdir=tmpdir
    )
    out = np.empty((B * S, N), dtype=np.float32)
    for c in range(NCORES):
        ms, nh = c // NSHARDS, c % NSHARDS
        out[ms * M_CORE : (ms + 1) * M_CORE, nh * N_CORE : (nh + 1) * N_CORE] = (
            res.results[c]["out"]
        )
    return out.reshape(B, S, N), res


def kernel(x, weight):
    out, _ = _run(x, weight, trace=False)
    return out


def run_traced(x, weight, tmpdir=None):
    """For test harnesses: returns (out, exec_time_ns)."""
    out, res = _run(x, weight, trace=True, tmpdir=tmpdir)
    return out, res.exec_time_ns


# revision 3
# speedup vs baseline: 1.1948x; 1.1948x over previous
"""FP8-quantized dense MLP (scaled matmul) on 8 Trainium2 NeuronCores.

Reference computation:
    x  [8, 2048, 4096] f32, weight [4096, 4096] f32
    sx = 448 / amax(|x|); sw = 448 / amax(|w|)
    out = (q8(x*sx) @ q8(w*sw)) * (1/sx) * (1/sw)     (q8 = OCP e4m3fn RNE)

Sharding: 4 M-shards x 2 N-shards over 8 cores (core c -> rows
[c//2*4096, +4096), cols [c%2*2048, +2048)).  Scales + fp8 quantization run
on host (O(MK+KN) elementwise prep); the O(MKN) matmul runs on device.

TRN2's FP8_EXP4 has max +-240 (OCP e4m3fn has +-448), so OCP-quantized values
256..448 would be NaN/Inf on device.  We therefore quantize to the OCP grid
*halved* (exact in fp8 for all but deep-subnormal values) by scaling with
sx/2 and clipping to +-224, and compensate with a *4 factor folded into the
output scale.  The device matmul (fp8 products, f32 accumulate) is then
bit-equivalent to the reference modulo f32 summation order.

Device kernel per core: out[4096, 2048] = xT.T @ w in fp8 DoubleRow mode
(K-tiles of 256); 216ns/MM warm = the N=512 DoubleRow stream roofline, so
exec = T_first_MM + 2048*216ns + prologue stalls/cold + tail and the
optimizations all attack the edges.

Hard-won constraints (measured on HW, do not regress):
  - The DoubleRow rhs pair-stride must be 2048B: w tiles [P, 2, N_CORE]
    sliced [:, :, n*512:+512].  An n-major layout (pair stride 512B) slowed
    every matmul 216->259ns (SBUF banking conflict between the two
    DoubleRow row-streams).
  - DMA packets below 4KB/partition-row run at ~120GB/s vs ~330GB/s for
    4KB rows, so input tiles stream whole (x0/w0 split pieces are a net
    loss on the one input queue; program order == arrival order).
  - All DMA triggers stay on nc.sync (gpsimd routing measured a 95us
    regression in a previous session).

Structure:
  - Input stream order [x0, w0, x1, w1..w15]: first MM at ~10.7us, m1
    work arrives at ~12.3us, stream saturates when w1 lands (~13.8us).
  - The n=3 PSUM bank alternates between ps3 and the warm-up bank (ps3b)
    across m-tiles, so consecutive m-tiles never serialize on a bank
    eviction: during the w-stream the PE has 8 MMs per 512KB w-tile
    (1.73us work per 1.55us wire cadence) and builds backlog.
  - Dummy warm-up/filler matmuls (N=256, on ps3b before m1 claims it)
    bridge the two ~0.7us prologue stalls; idle windows re-throttle the
    HAM clock gate to 1.2GHz (the 467us baseline lost ~5us to that).
  - The last m-tile runs n-outer/k2-inner so banks ps0..ps2 finish,
    evict and DMA out under the remaining MM stream; only ps3's
    eviction+256KB DMA trail the last MM.
"""

import numpy as np
import ml_dtypes

FP8_MAX = 448.0
B, S, K, N = 8, 2048, 4096, 4096
NCORES = 8
MSHARDS = 4
NSHARDS = 2
M_CORE = B * S // MSHARDS   # 4096 rows per core
N_CORE = N // NSHARDS       # 2048 cols per core
P = 128
K2 = K // 256    # 16 DoubleRow k-tiles of 256
MT = M_CORE // P  # 32 m-tiles per core
NFREE = 512      # matmul free dim == one PSUM bank of f32
NT = N_CORE // NFREE  # 4 PSUM banks per m-tile

# Filler counts (N=256 dummy matmuls, ~107ns warm / ~213ns cold each):
# F0 covers PE idle from the preamble (~6.3us) to the first real MM
# (~10.7us); F1/F2 bridge the stalls before x1/w1 land.
F0 = 20
F1 = 6
F2 = 6
F3 = 2

_E4M3 = ml_dtypes.float8_e4m3  # TRN semantics: max +-240

_nc_cache = None


def _build_nc():
    from concourse import bacc, tile, mybir

    nc = bacc.Bacc("TRN2", debug=False)
    xt_d = nc.dram_tensor("xt", [MT, P, K2, 2, P], mybir.dt.float8e4, kind="ExternalInput")
    wt_d = nc.dram_tensor(
        "wt", [K2, P, 2, N_CORE], mybir.dt.float8e4, kind="ExternalInput"
    )
    sc_d = nc.dram_tensor("sc", [P, 1], mybir.dt.float32, kind="ExternalInput")
    out_d = nc.dram_tensor("out", [M_CORE, N_CORE], mybir.dt.float32, kind="ExternalOutput")

    with tile.TileContext(nc) as tc:
        with (
            tc.tile_pool(name="wp", bufs=1) as wp,
            tc.tile_pool(name="xp", bufs=4) as xp,
            tc.tile_pool(name="op", bufs=4) as op,
            tc.tile_pool(name="cp", bufs=1) as cp,
            tc.tile_pool(name="pp", bufs=2, space="PSUM") as pp,
        ):
            # Fillers write the ps3b bank, which odd m-tiles later claim
            # for their n=3 accumulator (WAR-tracked by Tile).  Sharing a
            # bank with a LIVE accumulator tag crashes (PSUM_COLLISION);
            # serial reuse is fine.
            wa = cp.tile([P, 2, P], mybir.dt.float8e4, tag="wa")
            wb = cp.tile([P, 2, 2 * P], mybir.dt.float8e4, tag="wb")
            nc.vector.memset(wa[:], 0)
            nc.vector.memset(wb[:], 0)
            psw = pp.tile([P, NFREE], mybir.dt.float32, tag="ps3b", bufs=1, name="psw")

            def filler(count):
                for _ in range(count):
                    nc.tensor.matmul(
                        psw[:, 0 : 2 * P],
                        wa[:],
                        wb[:],
                        start=True,
                        stop=True,
                        perf_mode=mybir.MatmulPerfMode.DoubleRow,
                    )

            filler(F0)

            sc_sb = cp.tile([P, 1], mybir.dt.float32, tag="sc")

            x_tiles = {
                0: xp.tile([P, K2, 2, P], mybir.dt.float8e4, tag="x", name="x0"),
                1: xp.tile([P, K2, 2, P], mybir.dt.float8e4, tag="x", name="x1"),
            }
            nc.sync.dma_start(x_tiles[0][:], xt_d[0])

            w_sb = []
            for k2 in range(K2):
                w_t = wp.tile([P, 2, N_CORE], mybir.dt.float8e4, tag=f"w{k2}")
                nc.sync.dma_start(w_t[:], wt_d[k2])
                w_sb.append(w_t)
                if k2 == 0:
                    nc.sync.dma_start(x_tiles[1][:], xt_d[1])
                if k2 == 8:
                    nc.sync.dma_start(sc_sb[:], sc_d[:])

            def alloc_ps(m):
                # n=3 alternates ps3 / ps3b so consecutive m-tiles never
                # wait on each other's bank eviction; ps0-2 double-buffer.
                # 2*3 + 1 + 1 = 8 banks.
                return [
                    pp.tile(
                        [P, NFREE],
                        mybir.dt.float32,
                        tag=("ps3" if m % 2 == 0 else "ps3b") if n == NT - 1 else f"ps{n}",
                        name=f"ps{m}_{n}",
                        bufs=1 if n == NT - 1 else 2,
                    )
                    for n in range(NT)
                ]

            def evict_one(m, n, bank):
                o_t = op.tile([P, NFREE], mybir.dt.float32, tag="o", name=f"o{m}_{n}")
                if n % 2 == 0:
                    nc.scalar.activation(
                        o_t[:],
                        bank[:],
                        mybir.ActivationFunctionType.Copy,
                        scale=sc_sb[:],
                    )
                else:
                    nc.vector.tensor_scalar_mul(o_t[:], bank[:], sc_sb[:])
                nc.sync.dma_start(
                    out_d[m * P : (m + 1) * P, n * NFREE : (n + 1) * NFREE],
                    o_t[:],
                )

            for m in range(MT):
                if m in x_tiles:
                    x_t = x_tiles.pop(m)
                else:
                    x_t = xp.tile([P, K2, 2, P], mybir.dt.float8e4, tag="x", name=f"x{m}")
                    nc.sync.dma_start(x_t[:], xt_d[m])
                ps = alloc_ps(m)
                if m < MT - 1:
                    for k2 in range(K2):
                        for n in range(NT):
                            nc.tensor.matmul(
                                ps[n][:],
                                x_t[:, k2],
                                w_sb[k2][:, :, n * NFREE : (n + 1) * NFREE],
                                start=(k2 == 0),
                                stop=(k2 == K2 - 1),
                                perf_mode=mybir.MatmulPerfMode.DoubleRow,
                            )
                        if m == 0 and k2 == 0:
                            filler(F1)
                        elif m == 0 and k2 == 1:
                            filler(F2)
                        elif m == 0 and k2 == 2:
                            filler(F3)
                    for n in range(NT):
                        evict_one(m, n, ps[n])
                else:
                    # Last m-tile n-outer: each bank finishes, evicts and
                    # DMAs out under the remaining banks' MM stream.
                    for n in range(NT):
                        for k2 in range(K2):
                            nc.tensor.matmul(
                                ps[n][:],
                                x_t[:, k2],
                                w_sb[k2][:, :, n * NFREE : (n + 1) * NFREE],
                                start=(k2 == 0),
                                stop=(k2 == K2 - 1),
                                perf_mode=mybir.MatmulPerfMode.DoubleRow,
                            )
                        evict_one(m, n, ps[n])

    nc.finalize()
    return nc


def _get_nc():
    global _nc_cache
    if _nc_cache is None:
        _nc_cache = _build_nc()
    return _nc_cache


def _amax(a):
    # max(|a|) without a full |a| temp; exact (max/min are exact in f32)
    return np.float32(max(np.float32(a.max()), -np.float32(a.min())))


def _prep(x, weight):
    """Host prep: scales, halved OCP-grid fp8 quantization, tiled layouts."""
    x = np.asarray(x, dtype=np.float32)
    weight = np.asarray(weight, dtype=np.float32)

    sx = np.float32(FP8_MAX) / np.maximum(_amax(x), np.float32(1e-12))
    sw = np.float32(FP8_MAX) / np.maximum(_amax(weight), np.float32(1e-12))
    clip = np.float32(FP8_MAX / 2.0)  # 224

    # weight: [K, N] -> per N-shard [K2, P, 2, N_CORE]:
    #   wt[k2, ki, o, n] = wq[k2*256 + o*128 + ki, nh*N_CORE + n]
    wbuf = weight * (sw * np.float32(0.5))
    np.clip(wbuf, -clip, clip, out=wbuf)
    wq = wbuf.astype(_E4M3)
    wts = [
        np.ascontiguousarray(
            wq[:, nh * N_CORE : (nh + 1) * N_CORE]
            .reshape(K2, 2, P, N_CORE)
            .transpose(0, 2, 1, 3)
        )
        for nh in range(NSHARDS)
    ]

    # x per M-shard ms: rows [ms*4096, +4096) -> [MT, P, K2, 2, P] with
    # xt[m, ki, k2, o, j] = xq[m*128+j, k2*256 + o*128 + ki]
    x2 = x.reshape(B * S, K)
    xts = []
    for ms in range(MSHARDS):
        xbuf = x2[ms * M_CORE : (ms + 1) * M_CORE] * (sx * np.float32(0.5))
        np.clip(xbuf, -clip, clip, out=xbuf)
        xq = xbuf.astype(_E4M3)
        xts.append(
            np.ascontiguousarray(xq.reshape(MT, P, K2, 2, P).transpose(0, 4, 2, 3, 1))
        )

    # output scale: psum = ref_matmul / 4  ->  multiply by 4 * (1/sx) * (1/sw)
    c = np.float32(4.0) * (np.float32(1.0) / sx) * (np.float32(1.0) / sw)
    sc = np.full((P, 1), c, dtype=np.float32)
    return xts, wts, sc


def _run(x, weight, trace=False, tmpdir=None):
    from concourse.bass_utils import run_bass_kernel_spmd

    nc = _get_nc()
    xts, wts, sc = _prep(x, weight)
    in_maps = [
        {"xt": xts[c // NSHARDS], "wt": wts[c % NSHARDS], "sc": sc}
        for c in range(NCORES)
    ]
    res = run_bass_kernel_spmd(
        nc, in_maps, list(range(NCORES)), trace=trace, tmpdir=tmpdir
    )
    out = np.empty((B * S, N), dtype=np.float32)
    for c in range(NCORES):
        ms, nh = c // NSHARDS, c % NSHARDS
        out[ms * M_CORE : (ms + 1) * M_CORE, nh * N_CORE : (nh + 1) * N_CORE] = (
            res.results[c]["out"]
        )
    return out.reshape(B, S, N), res


def kernel(x, weight):
    out, _ = _run(x, weight, trace=False)
    return out


def run_traced(x, weight, tmpdir=None):
    """For test harnesses: returns (out, exec_time_ns)."""
    out, res = _run(x, weight, trace=True, tmpdir=tmpdir)
    return out, res.exec_time_ns
